# revision 2
# baseline (speedup 1.0000x reference)
"""GATConv x2 + LayerNorm (GNN message passing) on 8 TRN2 NeuronCores.

Strategy (edge-parallel, dst-sharded), v2:
  - Nodes are LPT-assigned to 8 cores balancing in-degree; each core owns
    all (non-self) edges whose dst it owns.  Self-loop edges are handled
    analytically in the window epilogues (no gather).
  - Host assigns each core's nodes to 128-slot "windows" balancing
    in-degree, sorts windows by edge count (desc) so per-window-index
    block counts align across cores, and lays out edges per window in
    128-edge blocks split low/high by source table row (int16 indices).
    Block counts KL[w]/KH[w] vary per window; padding slots use idx=-1
    (gather skips them) after a cross-core count-equalizing dummy fill.
  - Phase 0: h1 = x @ W1 for own nodes + attention dots; packed rows
    [h1 bf16 | e_src f32] -> AllGather.  e_dst (and e_src for the self
    term) stay in SBUF-resident tiles.
  - Edge phases: dma_gather source rows; e_dst broadcast to edge slots
    via per-block transposed one-hot (fp8, host-built) matmuls against
    the window's e_dst tile; exp(leaky_relu(e)) weights; one matmul per
    128-edge block with a one-hot dst matrix accumulates weighted
    messages + softmax denominators in PSUM.  Epilogues add the exact
    self-loop term, normalize, and either build table 2 (relu, @W2,
    attention dots -> AllGather) or finish (head-mean, bias, LayerNorm).
"""

import sys

sys.path.insert(0, "/opt/trn_rl_repo")

import math
import os
import numpy as np
import ml_dtypes

import concourse.bass as bass
import concourse.bacc as bacc
import concourse.mybir as mybir
from concourse import tile
from concourse.bass_utils import run_bass_kernel_spmd

F32 = mybir.dt.float32
BF16 = mybir.dt.bfloat16
FP8 = mybir.dt.float8e4
I16 = mybir.dt.int16
I32 = mybir.dt.int32
AF = mybir.ActivationFunctionType
ALU = mybir.AluOpType

CORES = 8
WIN = 128
H = 4

NEG_SLOPE = 0.2
EPS_LN = 1e-5
E_CLAMP = 60.0


class Cfg:
    def __init__(self, N, IN_DIM, C1, C2, KLs, KHs, regs_lo, regs_hi):
        assert N % CORES == 0
        self.N = N
        self.IN = IN_DIM
        self.C1 = C1
        self.C2 = C2
        self.F1 = H * C1          # 256
        self.F2 = H * C2          # 512
        self.NODES_PC = N // CORES
        self.NW = math.ceil(self.NODES_PC / WIN)
        self.SLOTS_PC = self.NW * WIN
        self.TOT = CORES * self.SLOTS_PC
        assert self.TOT % 2 == 0
        self.HALF = self.TOT // 2
        assert self.HALF <= 32767, self.HALF
        assert self.SLOTS_PC <= 32767
        self.KLs = KLs            # per-window low-half block counts
        self.KHs = KHs
        self.Ks = [a + b for a, b in zip(KLs, KHs)]
        self.regs_lo = regs_lo    # per-window valid idx count (16-aligned)
        self.regs_hi = regs_hi
        self.oKL = np.concatenate([[0], np.cumsum(KLs)]).tolist()
        self.oKH = np.concatenate([[0], np.cumsum(KHs)]).tolist()
        self.oK = np.concatenate([[0], np.cumsum(self.Ks)]).tolist()
        self.SKL = self.oKL[-1]
        self.SKH = self.oKH[-1]
        self.SK = self.oK[-1]
        self.KLmax = max(KLs) if KLs else 0
        self.KHmax = max(KHs) if KHs else 0
        self.Kmax = max(self.Ks) if self.Ks else 0
        self.R1 = _pad_row(self.F1 * 2 + 16)   # bf16 cols
        self.R2 = _pad_row(self.F2 * 2 + 16)
        self.key = (N, IN_DIM, C1, C2, tuple(KLs), tuple(KHs),
                    tuple(regs_lo), tuple(regs_hi))


def _pad_row(nbytes):
    """Round row bytes up to a multiple of 256; return bf16 col count."""
    b = ((nbytes + 255) // 256) * 256
    return b // 2


def _wrap_idx(flat):
    """[L] int (L%16==0) -> [128, L/16] int16 wrapped+replicated."""
    L = len(flat)
    assert L % 16 == 0
    w = flat.reshape(L // 16, 16).T          # [16, L/16]
    w = np.concatenate([w] * 8, axis=0)      # [128, L/16]
    return np.ascontiguousarray(w.astype(np.int16))


def _lpt_bins(loads, nbins, cap):
    """Greedy LPT: place items (desc by load) into least-loaded bin with
    count < cap.  Returns (bin_of, off_of) arrays."""
    import heapq
    order = np.argsort(-loads, kind="stable")
    heap = [(0, 0, b) for b in range(nbins)]
    heapq.heapify(heap)
    bin_of = np.empty(len(loads), dtype=np.int64)
    off_of = np.empty(len(loads), dtype=np.int64)
    for i in order:
        while True:
            load, cnt, b = heapq.heappop(heap)
            if cnt < cap:
                break
        bin_of[i] = b
        off_of[i] = cnt
        heapq.heappush(heap, (load + loads[i], cnt + 1, b))
    return bin_of, off_of


def prep(x, edge_index, W1, a_src1, a_dst1, b1, W2, a_src2, a_dst2, b2,
         gamma, beta):
    """Host-side sharding. Returns (cfg, in_maps, slot_global)."""
    N, IN_DIM = x.shape
    C1 = a_src1.shape[1]
    C2 = a_src2.shape[1]

    src = np.asarray(edge_index[0], dtype=np.int64)
    dst = np.asarray(edge_index[1], dtype=np.int64)

    NODES_PC = N // CORES
    NW = math.ceil(NODES_PC / WIN)
    SLOTS_PC = NW * WIN
    HALF = CORES * SLOTS_PC // 2

    deg = np.bincount(dst, minlength=N)

    # ---- assign nodes to cores (balance in-degree), then to windows ----
    core_of, _ = _lpt_bins(deg, CORES, NODES_PC)
    slot_global = np.empty(N, dtype=np.int64)
    win_of = np.empty(N, dtype=np.int64)
    off_of = np.empty(N, dtype=np.int64)
    core_nodes = []
    for c in range(CORES):
        nodes = np.nonzero(core_of == c)[0]
        core_nodes.append(nodes)
        wsel, osel = _lpt_bins(deg[nodes], NW, WIN)
        win_of[nodes] = wsel
        off_of[nodes] = osel

    # per (core, window) lo/hi counts for sorting + alignment
    owner = core_of[dst]
    # provisional slot (pre-sort) to classify lo/hi
    prov_slot = core_of * SLOTS_PC + win_of * WIN + off_of
    e_lo = prov_slot[src] < HALF
    cnt_lo = np.zeros((CORES, NW), dtype=np.int64)
    cnt_hi = np.zeros((CORES, NW), dtype=np.int64)
    for c in range(CORES):
        m = owner == c
        w = win_of[dst[m]]
        lo = e_lo[m]
        cnt_lo[c] = np.bincount(w[lo], minlength=NW)
        cnt_hi[c] = np.bincount(w[~lo], minlength=NW)

    # sort windows per core by total count desc; window index -> rank
    rank_of = np.empty((CORES, NW), dtype=np.int64)
    for c in range(CORES):
        order = np.argsort(-(cnt_lo[c] + cnt_hi[c]), kind="stable")
        rank_of[c, order] = np.arange(NW)
        cnt_lo[c] = cnt_lo[c][order]
        cnt_hi[c] = cnt_hi[c][order]
    win_of = rank_of[core_of, win_of]
    slot_global = core_of * SLOTS_PC + win_of * WIN + off_of

    # NOTE: lo/hi classification used PRE-sort slots; recompute post-sort.
    e_lo = slot_global[src] < HALF
    for c in range(CORES):
        m = owner == c
        w = win_of[dst[m]]
        lo = e_lo[m]
        cnt_lo[c] = np.bincount(w[lo], minlength=NW)
        cnt_hi[c] = np.bincount(w[~lo], minlength=NW)

    # cross-core maxima, 16-aligned valid counts, block counts
    max_lo = cnt_lo.max(axis=0)
    max_hi = cnt_hi.max(axis=0)
    regs_lo = (np.ceil(max_lo / 16) * 16).astype(np.int64)
    regs_hi = (np.ceil(max_hi / 16) * 16).astype(np.int64)
    KLs = np.ceil(regs_lo / WIN).astype(np.int64)
    KHs = np.ceil(regs_hi / WIN).astype(np.int64)
    regs_lo = np.minimum(regs_lo, KLs * WIN)
    regs_hi = np.minimum(regs_hi, KHs * WIN)

    cfg = Cfg(N, IN_DIM, C1, C2, KLs.tolist(), KHs.tolist(),
              regs_lo.tolist(), regs_hi.tolist())

    src_row = slot_global[src]
    e_w = win_of[dst]
    e_off = off_of[dst]

    in_maps = []
    for c in range(CORES):
        m = owner == c
        ew, eo, er, el = e_w[m], e_off[m], src_row[m], e_lo[m]
        idx_lo = np.empty(cfg.SKL * WIN, dtype=np.int64)
        idx_hi = np.empty(cfg.SKH * WIN, dtype=np.int64)
        st = np.zeros((128, cfg.SK * WIN), dtype=np.float32)
        dstoffT = np.full((128, cfg.SK), 999.0, dtype=np.float32)
        for w in range(NW):
            KL, KH_, K = cfg.KLs[w], cfg.KHs[w], cfg.Ks[w]
            wm = ew == w
            lo_m = wm & el
            hi_m = wm & ~el
            rlo, olo = er[lo_m], eo[lo_m]
            rhi, ohi = er[hi_m] - HALF, eo[hi_m]

            def fill(idx_arr, base, nblk, reg, rows):
                n = len(rows)
                sl = idx_arr[base: base + nblk * WIN]
                sl[:n] = rows
                sl[n:reg] = 0          # dummy valid (row 0)
                sl[reg:] = -1          # skipped by gather
                return n

            fill(idx_lo, cfg.oKL[w] * WIN, KL, cfg.regs_lo[w], rlo)
            fill(idx_hi, cfg.oKH[w] * WIN, KH_, cfg.regs_hi[w], rhi)

            # dst offsets per edge slot, block order [lo | hi]
            doff = np.full(K * WIN, 999.0, dtype=np.float32)
            doff[: len(olo)] = olo
            doff[KL * WIN: KL * WIN + len(ohi)] = ohi
            d2 = doff.reshape(K, WIN)
            dstoffT[:, cfg.oK[w]: cfg.oK[w] + K] = d2.T
            # transposed one-hot: st[d, block*128+j] = (doff[block,j]==d)
            blk = st[:, cfg.oK[w] * WIN: (cfg.oK[w] + K) * WIN]
            valid = doff < 128
            jj = np.nonzero(valid)[0]
            blk[doff[jj].astype(np.int64), jj] = 1.0

        xs = np.zeros((SLOTS_PC, IN_DIM), dtype=np.float32)
        nodes = core_nodes[c]
        loc = win_of[nodes] * WIN + off_of[nodes]
        xs[loc] = np.asarray(x[nodes], dtype=np.float32)

        in_maps.append({
            "xT": np.ascontiguousarray(xs.T),
            "idx_lo": _wrap_idx(idx_lo),
            "idx_hi": _wrap_idx(idx_hi),
            "stT": np.ascontiguousarray(st.astype(ml_dtypes.float8_e4m3fn)),
            "dstoffT": np.ascontiguousarray(dstoffT),
            "W1": np.asarray(W1, np.float32),
            "W2": np.asarray(W2, np.float32),
            "asrc1": np.asarray(a_src1, np.float32).reshape(1, -1),
            "adst1": np.asarray(a_dst1, np.float32).reshape(1, -1),
            "asrc2": np.asarray(a_src2, np.float32).reshape(1, -1),
            "adst2": np.asarray(a_dst2, np.float32).reshape(1, -1),
            "b1": np.asarray(b1, np.float32).reshape(1, -1),
            "b2": np.asarray(b2, np.float32).reshape(1, -1),
            "gamma": np.asarray(gamma, np.float32).reshape(1, -1),
            "beta": np.asarray(beta, np.float32).reshape(1, -1),
        })
    return cfg, in_maps, slot_global


# --------------------------------------------------------------------------
# device program
# --------------------------------------------------------------------------

def build(cfg):
    PH = os.environ.get("GAT_PHASES", "012")
    _sc = int(os.environ.get("GAT_SCRATCH", "16384"))
    nc = bacc.Bacc("TRN2", target_bir_lowering=False, debug=False,
                   num_devices=CORES, dynamic_dma_scratch_size=_sc)
    NW = cfg.NW
    F1, F2, R1, R2 = cfg.F1, cfg.F2, cfg.R1, cfg.R2
    C1, C2 = cfg.C1, cfg.C2
    SL, TOT, HALF = cfg.SLOTS_PC, cfg.TOT, cfg.HALF
    RG = [list(range(CORES))]

    # ---- kernel I/O ----
    xT = nc.dram_tensor("xT", [cfg.IN, SL], F32, kind="ExternalInput")
    idx_lo = nc.dram_tensor("idx_lo", [128, cfg.SKL * 8], I16,
                            kind="ExternalInput")
    idx_hi = nc.dram_tensor("idx_hi", [128, cfg.SKH * 8], I16,
                            kind="ExternalInput")
    stT = nc.dram_tensor("stT", [128, cfg.SK * WIN], FP8,
                         kind="ExternalInput")
    dstoffT = nc.dram_tensor("dstoffT", [128, cfg.SK], F32,
                             kind="ExternalInput")
    W1 = nc.dram_tensor("W1", [cfg.IN, F1], F32, kind="ExternalInput")
    W2 = nc.dram_tensor("W2", [F1, F2], F32, kind="ExternalInput")
    vecs = {}
    for nm, d in [("asrc1", F1), ("adst1", F1), ("asrc2", F2), ("adst2", F2),
                  ("b1", F1), ("b2", C2), ("gamma", C2), ("beta", C2)]:
        vecs[nm] = nc.dram_tensor(nm, [1, d], F32, kind="ExternalInput")
    out = nc.dram_tensor("out", [SL, C2], F32, kind="ExternalOutput")

    # ---- internal DRAM ----
    t1_shard = nc.dram_tensor("t1_shard", [SL, R1], BF16, kind="Internal")
    t2_shard = nc.dram_tensor("t2_shard", [SL, R2], BF16, kind="Internal")
    t1_full = nc.dram_tensor("t1_full", [TOT, R1], BF16, kind="Internal",
                             addr_space="Shared")
    t2_full = nc.dram_tensor("t2_full", [TOT, R2], BF16, kind="Internal",
                             addr_space="Shared")

    with tile.TileContext(nc) as tc:
        with tc.tile_pool(name="const", bufs=1) as cp:
            # iota row [128, Kmax*128] f32 (value = col % 128)
            iota_i = cp.tile([128, cfg.Kmax * 128], I32)
            nc.gpsimd.iota(iota_i[:], pattern=[[0, cfg.Kmax], [1, 128]],
                           base=0, channel_multiplier=0)
            iota_f = cp.tile([128, cfg.Kmax * 128], F32)
            nc.vector.tensor_copy(iota_f[:], iota_i[:])
            ic_i = cp.tile([128, 1], I32)
            nc.gpsimd.iota(ic_i[:], pattern=[[0, 1]], base=0,
                           channel_multiplier=1)
            ic_f = cp.tile([128, 1], F32)
            nc.vector.tensor_copy(ic_f[:], ic_i[:])
            ident = cp.tile([128, 128], F32)
            nc.vector.tensor_scalar(ident[:], iota_f[:, 0:128], ic_f[:, 0:1],
                                    None, ALU.is_equal)

            W1sb = cp.tile([128, F1], F32)
            nc.sync.dma_start(W1sb[:], W1[:, :])
            W2a = cp.tile([128, F2], F32)
            W2b = cp.tile([128, F2], F32)
            nc.sync.dma_start(W2a[:], W2[0:128, :])
            nc.sync.dma_start(W2b[:], W2[128:256, :])
            dstoffT_sb = cp.tile([128, cfg.SK], F32)
            nc.sync.dma_start(dstoffT_sb[:], dstoffT[:, :])

            ones = cp.tile([1, 128], F32)
            nc.vector.memset(ones[:], 1.0)
            epsb = cp.tile([128, 1], F32)
            nc.vector.memset(epsb[:], EPS_LN)

            # SBUF-resident per-node attention values (window-slot layout):
            # [128, NW*H]: es/ed for layer 1 and 2 (f32) + bf16 ed copies
            # used as the moving operand of the fp8 one-hot broadcasts.
            es1_sb = cp.tile([128, NW * H], F32)
            ed1_sb = cp.tile([128, NW * H], F32)
            es2_sb = cp.tile([128, NW * H], F32)
            ed2_sb = cp.tile([128, NW * H], F32)
            ed1_bf = cp.tile([128, NW * H], BF16)
            ed2_bf = cp.tile([128, NW * H], BF16)

            # broadcast small vectors to [128, D] via 1-row matmul
            bc = {}
            with tc.tile_pool(name="bcp", bufs=2, space="PSUM") as bps, \
                 tc.tile_pool(name="bcs", bufs=1) as bsb:
                for nm, d in [("asrc1", F1), ("adst1", F1), ("asrc2", F2),
                              ("adst2", F2), ("b1", F1), ("b2", C2),
                              ("gamma", C2), ("beta", C2)]:
                    vsb = bsb.tile([1, d], F32, tag="vload")
                    nc.sync.dma_start(vsb[:], vecs[nm][:, :])
                    t = cp.tile([128, d], F32, tag=f"bc_{nm}")
                    ps = bps.tile([128, d], F32, tag="bcps")
                    nc.tensor.matmul(ps[:], ones[:], vsb[:], start=True,
                                     stop=True)
                    nc.vector.tensor_copy(t[:], ps[:])
                    bc[nm] = t

            # ================= Phase 0: node tables =================
            REPEAT = int(os.environ.get("GAT_REPEAT", "1"))
            for _rep in range(REPEAT):
              with tc.tile_pool(name="p0", bufs=3) as p0, \
                   tc.tile_pool(name="p0ps", bufs=2, space="PSUM") as p0ps:
                  for w in range(NW):
                      xt = p0.tile([128, 128], F32, tag="xt")
                      nc.sync.dma_start(xt[:], xT[:, w * WIN:(w + 1) * WIN])
                      h1 = p0ps.tile([128, F1], F32, tag="h1")
                      nc.tensor.matmul(h1[:], xt[:], W1sb[:], start=True,
                                       stop=True)
                      prod = p0.tile([128, F1], F32, tag="prod")
                      es1 = es1_sb[:, w * H:(w + 1) * H]
                      nc.vector.tensor_tensor(prod[:], h1[:], bc["asrc1"][:],
                                              ALU.mult)
                      nc.vector.reduce_sum(
                          es1, prod[:].rearrange("p (h c) -> p h c", c=C1),
                          axis=mybir.AxisListType.X)
                      prod2 = p0.tile([128, F1], F32, tag="prod2")
                      ed1 = ed1_sb[:, w * H:(w + 1) * H]
                      nc.vector.tensor_tensor(prod2[:], h1[:], bc["adst1"][:],
                                              ALU.mult)
                      nc.vector.reduce_sum(
                          ed1, prod2[:].rearrange("p (h c) -> p h c", c=C1),
                          axis=mybir.AxisListType.X)
                      nc.vector.tensor_copy(ed1_bf[:, w * H:(w + 1) * H], ed1)
                      pk = p0.tile([128, R1], BF16, tag="pk")
                      nc.scalar.copy(pk[:, 0:F1], h1[:])
                      nc.vector.tensor_copy(
                          pk[:, F1:F1 + 8].bitcast(F32), es1)
                      nc.sync.dma_start(
                          t1_shard[w * WIN:(w + 1) * WIN, 0:F1 + 8],
                          pk[:, 0:F1 + 8])

              if "1" in PH or "2" in PH or "g" in PH:
                  nc.gpsimd.collective_compute(
                      "AllGather", ALU.bypass, replica_groups=RG,
                      ins=[t1_shard[:, :]], outs=[t1_full[:, :]])

              # ================= Phase 1 and 2 =================
              def edge_phase(layer):
                  F = F1 if layer == 1 else F2
                  C = C1 if layer == 1 else C2
                  R = R1 if layer == 1 else R2
                  tfull = t1_full if layer == 1 else t2_full
                  tshard = t1_shard if layer == 1 else t2_shard
                  es_sb = es1_sb if layer == 1 else es2_sb
                  ed_sb = ed1_sb if layer == 1 else ed2_sb
                  ed_bf = ed1_bf if layer == 1 else ed2_bf
                  sfx = f"L{layer}"
                  _nb = int(os.environ.get("GAT_BUFS", "2"))
                  _nwl = int(os.environ.get("GAT_NWLIM", str(NW)))
                  GMAX = int(os.environ.get("GAT_GMAX", "4"))
                  KLm, KHm, Km = cfg.KLmax, cfg.KHmax, cfg.Kmax
                  with tc.tile_pool(name=f"pe{sfx}", bufs=_nb) as pe, \
                       tc.tile_pool(name=f"peps{sfx}", bufs=_nb,
                                    space="PSUM") as pps, \
                       tc.tile_pool(name=f"po{sfx}", bufs=_nb) as po, \
                       tc.tile_pool(name=f"pops{sfx}", bufs=_nb,
                                    space="PSUM") as ops:
                      for w in range(min(NW, _nwl)):
                          KL, KH_, K = cfg.KLs[w], cfg.KHs[w], cfg.Ks[w]
                          glo = pe.tile([128, KLm * R], BF16, tag="glo")
                          ghi = pe.tile([128, KHm * R], BF16, tag="ghi")
                          if w < _nb:
                              # first touch of each rotating buffer: clear
                              # so skipped (-1) slots never hold NaN bits
                              nc.vector.memset(glo[:], 0.0)
                              nc.vector.memset(ghi[:], 0.0)
                          ilo = pe.tile([128, KLm * 8], I16, tag="ilo")
                          nc.sync.dma_start(
                              ilo[:, 0:KL * 8],
                              idx_lo[:, cfg.oKL[w] * 8:
                                     (cfg.oKL[w] + KL) * 8])
                          ihi = pe.tile([128, KHm * 8], I16, tag="ihi")
                          nc.sync.dma_start(
                              ihi[:, 0:KH_ * 8],
                              idx_hi[:, cfg.oKH[w] * 8:
                                     (cfg.oKH[w] + KH_) * 8])
                          st_t = pe.tile([128, Km * WIN], FP8, tag="st")
                          nc.sync.dma_start(
                              st_t[:, 0:K * WIN],
                              stT[:, cfg.oK[w] * WIN:(cfg.oK[w] + K) * WIN])

                          def gcalls(tile_, src_ap, idx_tile, nblk, reg,
                                     elem):
                              gv = tile_[:].rearrange("p (t e) -> p t e",
                                                      e=elem)
                              for g0 in range(0, nblk, GMAX):
                                  nb = min(GMAX, nblk - g0)
                                  r = min(max(reg - g0 * WIN, 0), nb * WIN)
                                  nc.gpsimd.dma_gather(
                                      gv[:, g0:g0 + nb, :], src_ap,
                                      idx_tile[:, g0 * 8:(g0 + nb) * 8],
                                      nb * WIN, r, elem)

                          gcalls(glo, tfull[0:HALF, :], ilo, KL,
                                 cfg.regs_lo[w], R)
                          gcalls(ghi, tfull[HALF:TOT, :], ihi, KH_,
                                 cfg.regs_hi[w], R)

                          # e_dst broadcast: per block, transposed one-hot
                          # (fp8) x ed_w (bf16) -> [edge, H] in PSUM
                          psE = pps.tile([128, Km * H], F32, tag="psE")
                          edw = ed_bf[:, w * H:(w + 1) * H]
                          for b in range(K):
                              nc.tensor.matmul(
                                  psE[:, b * H:(b + 1) * H],
                                  st_t[:, b * WIN:(b + 1) * WIN],
                                  edw, start=True, stop=True)

                          e_all = pe.tile([128, Km * H], F32, tag="e_all")
                          ev = e_all[:].rearrange("p (k h) -> p k h", h=H)
                          psEv = psE[:].rearrange("p (k h) -> p k h", h=H)
                          if KL:
                              nc.vector.tensor_tensor(
                                  ev[:, 0:KL, :],
                                  glo[:, 0:KL * R]
                                  .rearrange("p (t e) -> p t e", e=R)
                                  [:, :, F:F + 8].bitcast(F32),
                                  psEv[:, 0:KL, :], ALU.add)
                          if KH_:
                              nc.vector.tensor_tensor(
                                  ev[:, KL:K, :],
                                  ghi[:, 0:KH_ * R]
                                  .rearrange("p (t e) -> p t e", e=R)
                                  [:, :, F:F + 8].bitcast(F32),
                                  psEv[:, KL:K, :], ALU.add)
                          # clamp (stale bits in skipped slots can be huge)
                          nc.vector.tensor_scalar(
                              e_all[:, 0:K * H], e_all[:, 0:K * H], E_CLAMP,
                              None, ALU.min)
                          e_sc = pe.tile([128, Km * H], F32, tag="e_sc")
                          nc.vector.tensor_scalar(
                              e_sc[:, 0:K * H], e_all[:, 0:K * H], NEG_SLOPE,
                              None, ALU.mult)
                          nc.vector.tensor_tensor(
                              e_all[:, 0:K * H], e_all[:, 0:K * H],
                              e_sc[:, 0:K * H], ALU.max)
                          w_all = pe.tile([128, Km * H], F32, tag="w_all")
                          nc.scalar.activation(w_all[:, 0:K * H],
                                               e_all[:, 0:K * H], AF.Exp)

                          S_all = pe.tile([128, Km * 128], BF16, tag="S_all")
                          nc.vector.tensor_tensor(
                              S_all[:, 0:K * 128]
                              .rearrange("p (k j) -> p k j", j=128),
                              iota_f[:, 0:K * 128]
                              .rearrange("p (k j) -> p k j", j=128),
                              dstoffT_sb[:, cfg.oK[w]:cfg.oK[w] + K]
                              .unsqueeze(-1).broadcast_to([128, K, 128]),
                              ALU.is_equal)

                          RC = F + H
                          rhs = pe.tile([128, Km * RC], BF16, tag="rhs")
                          rv = rhs[:].rearrange("p (k r) -> p k r", r=RC)
                          wv = w_all[:, 0:K * H].rearrange(
                              "p (k h) -> p k h", h=H)
                          if KL:
                              nc.vector.tensor_tensor(
                                  rv[:, 0:KL, 0:F].rearrange(
                                      "p k (h c) -> p k h c", c=C),
                                  glo[:, 0:KL * R]
                                  .rearrange("p (t e) -> p t e", e=R)
                                  [:, :, 0:F].rearrange(
                                      "p k (h c) -> p k h c", c=C),
                                  wv[:, 0:KL, :].unsqueeze(-1)
                                  .broadcast_to([128, KL, H, C]),
                                  ALU.mult)
                          if KH_:
                              nc.vector.tensor_tensor(
                                  rv[:, KL:K, 0:F].rearrange(
                                      "p k (h c) -> p k h c", c=C),
                                  ghi[:, 0:KH_ * R]
                                  .rearrange("p (t e) -> p t e", e=R)
                                  [:, :, 0:F].rearrange(
                                      "p k (h c) -> p k h c", c=C),
                                  wv[:, KL:K, :].unsqueeze(-1)
                                  .broadcast_to([128, KH_, H, C]),
                                  ALU.mult)
                          nc.vector.tensor_copy(rv[:, 0:K, F:F + H], wv)

                          # self-loop weight for this window (exact):
                          wself = po.tile([128, H], F32, tag="wself")
                          nc.vector.tensor_tensor(
                              wself[:], es_sb[:, w * H:(w + 1) * H],
                              ed_sb[:, w * H:(w + 1) * H], ALU.add)
                          wsc = po.tile([128, H], F32, tag="wsc")
                          nc.vector.tensor_scalar(wsc[:], wself[:], NEG_SLOPE,
                                                  None, ALU.mult)
                          nc.vector.tensor_tensor(wself[:], wself[:], wsc[:],
                                                  ALU.max)
                          nc.scalar.activation(wself[:], wself[:], AF.Exp)
                          # own h rows (for the self message term)
                          hw = po.tile([128, F], F32, tag="hw")
                          hwb = po.tile([128, F], BF16, tag="hwb")
                          nc.sync.dma_start(
                              hwb[:], tshard[w * WIN:(w + 1) * WIN, 0:F])
                          nc.scalar.copy(hw[:], hwb[:])
                          selfm = po.tile([128, F], F32, tag="selfm")
                          nc.vector.tensor_tensor(
                              selfm[:].rearrange("p (h c) -> p h c", c=C),
                              hw[:].rearrange("p (h c) -> p h c", c=C),
                              wself[:].unsqueeze(-1)
                              .broadcast_to([128, H, C]),
                              ALU.mult)

                          if layer == 1:
                              psW = pps.tile([128, F1 + H], F32, tag="psW")
                              for b in range(K):
                                  nc.tensor.matmul(
                                      psW[:], S_all[:, b * 128:(b + 1) * 128],
                                      rv[:, b, :], start=(b == 0),
                                      stop=(b == K - 1))
                              _epilogue1(nc, tc, po, ops, psW, bc, W2a, W2b,
                                         ident, cfg, w, t2_shard, wself,
                                         selfm, es2_sb, ed2_sb, ed2_bf)
                          else:
                              psA = pps.tile([128, F2], F32, tag="psA")
                              psD = pps.tile([128, H], F32, tag="psD")
                              for b in range(K):
                                  S_b = S_all[:, b * 128:(b + 1) * 128]
                                  nc.tensor.matmul(
                                      psA[:], S_b, rv[:, b, 0:F],
                                      start=(b == 0), stop=(b == K - 1))
                                  nc.tensor.matmul(
                                      psD[:], S_b, rv[:, b, F:F + H],
                                      start=(b == 0), stop=(b == K - 1))
                              _epilogue2(nc, po, psA, psD, bc, cfg, w, out,
                                         wself, selfm)

              def _epilogue1(nc, tc, po, ops, psW, bc, W2a, W2b, ident, cfg,
                             w, t2_shard, wself, selfm, es2_sb, ed2_sb,
                             ed2_bf):
                  den = po.tile([128, H], F32, tag="den")
                  nc.vector.tensor_tensor(den[:], psW[:, F1:F1 + H],
                                          wself[:], ALU.add)
                  rec = po.tile([128, H], F32, tag="rec")
                  nc.vector.reciprocal(rec[:], den[:])
                  num = po.tile([128, F1], F32, tag="num")
                  nc.vector.tensor_tensor(num[:], psW[:, 0:F1], selfm[:],
                                          ALU.add)
                  o1 = po.tile([128, F1], F32, tag="o1")
                  nc.vector.tensor_tensor(
                      o1[:].rearrange("p (h c) -> p h c", c=C1),
                      num[:].rearrange("p (h c) -> p h c", c=C1),
                      rec[:].unsqueeze(-1).broadcast_to([128, H, C1]),
                      ALU.mult)
                  nc.vector.tensor_tensor(o1[:], o1[:], bc["b1"][:], ALU.add)
                  nc.scalar.activation(o1[:], o1[:], AF.Relu)
                  tp = ops.tile([128, 128], F32, tag="tp")
                  t0 = po.tile([128, 128], F32, tag="t0")
                  nc.tensor.transpose(tp[:], o1[:, 0:128], ident[:])
                  nc.vector.tensor_copy(t0[:], tp[:])
                  tp2 = ops.tile([128, 128], F32, tag="tp")
                  t1t = po.tile([128, 128], F32, tag="t1t")
                  nc.tensor.transpose(tp2[:], o1[:, 128:256], ident[:])
                  nc.vector.tensor_copy(t1t[:], tp2[:])
                  h2 = ops.tile([128, F2], F32, tag="h2")
                  nc.tensor.matmul(h2[:], t0[:], W2a[:], start=True,
                                   stop=False)
                  nc.tensor.matmul(h2[:], t1t[:], W2b[:], start=False,
                                   stop=True)
                  pr = po.tile([128, F2], F32, tag="pr")
                  es2 = es2_sb[:, w * H:(w + 1) * H]
                  nc.vector.tensor_tensor(pr[:], h2[:], bc["asrc2"][:],
                                          ALU.mult)
                  nc.vector.reduce_sum(
                      es2, pr[:].rearrange("p (h c) -> p h c", c=C2),
                      axis=mybir.AxisListType.X)
                  pr2 = po.tile([128, F2], F32, tag="pr2")
                  ed2 = ed2_sb[:, w * H:(w + 1) * H]
                  nc.vector.tensor_tensor(pr2[:], h2[:], bc["adst2"][:],
                                          ALU.mult)
                  nc.vector.reduce_sum(
                      ed2, pr2[:].rearrange("p (h c) -> p h c", c=C2),
                      axis=mybir.AxisListType.X)
                  nc.vector.tensor_copy(ed2_bf[:, w * H:(w + 1) * H], ed2)
                  pk2 = po.tile([128, R2], BF16, tag="pk2")
                  nc.scalar.copy(pk2[:, 0:F2], h2[:])
                  nc.vector.tensor_copy(pk2[:, F2:F2 + 8].bitcast(F32), es2)
                  nc.sync.dma_start(t2_shard[w * WIN:(w + 1) * WIN, 0:F2 + 8],
                                    pk2[:, 0:F2 + 8])

              def _epilogue2(nc, po, psA, psD, bc, cfg, w, out, wself, selfm):
                  den = po.tile([128, H], F32, tag="den2")
                  nc.vector.tensor_tensor(den[:], psD[:], wself[:], ALU.add)
                  rec = po.tile([128, H], F32, tag="rec2")
                  nc.vector.reciprocal(rec[:], den[:])
                  num = po.tile([128, F2], F32, tag="num2")
                  nc.vector.tensor_tensor(num[:], psA[:], selfm[:], ALU.add)
                  tmp = po.tile([128, F2], F32, tag="tmp")
                  nc.vector.tensor_tensor(
                      tmp[:].rearrange("p (h c) -> p h c", c=C2),
                      num[:].rearrange("p (h c) -> p h c", c=C2),
                      rec[:].unsqueeze(-1).broadcast_to([128, H, C2]),
                      ALU.mult)
                  o2 = po.tile([128, C2], F32, tag="o2")
                  nc.vector.reduce_sum(
                      o2[:], tmp[:].rearrange("p (h c) -> p c h", c=C2),
                      axis=mybir.AxisListType.X)
                  nc.vector.tensor_scalar(o2[:], o2[:], 1.0 / H, None,
                                          ALU.mult)
                  nc.vector.tensor_tensor(o2[:], o2[:], bc["b2"][:], ALU.add)
                  mu = po.tile([128, 1], F32, tag="mu")
                  nc.vector.reduce_sum(mu[:], o2[:], axis=mybir.AxisListType.X)
                  nc.vector.tensor_scalar(mu[:], mu[:], 1.0 / C2, None,
                                          ALU.mult)
                  xc = po.tile([128, C2], F32, tag="xc")
                  nc.vector.tensor_scalar(xc[:], o2[:], mu[:], None,
                                          ALU.subtract)
                  sq = po.tile([128, C2], F32, tag="sq")
                  ssq = po.tile([128, 1], F32, tag="ssq")
                  nc.scalar.activation(sq[:], xc[:], AF.Square,
                                       accum_out=ssq[:])
                  sdev = po.tile([128, 1], F32, tag="sdev")
                  nc.scalar.activation(sdev[:], ssq[:], AF.Sqrt,
                                       scale=1.0 / C2, bias=epsb[:, 0:1])
                  rstd = po.tile([128, 1], F32, tag="rstd")
                  nc.vector.reciprocal(rstd[:], sdev[:])
                  xn = po.tile([128, C2], F32, tag="xn")
                  nc.vector.tensor_scalar(xn[:], xc[:], rstd[:], None,
                                          ALU.mult)
                  nc.vector.tensor_tensor(xn[:], xn[:], bc["gamma"][:],
                                          ALU.mult)
                  ot = po.tile([128, C2], F32, tag="ot")
                  nc.vector.tensor_tensor(ot[:], xn[:], bc["beta"][:],
                                          ALU.add)
                  nc.sync.dma_start(out[w * WIN:(w + 1) * WIN, :], ot[:])

              if "1" in PH:
                  edge_phase(1)
              if "2" in PH or "h" in PH:
                  nc.gpsimd.collective_compute(
                      "AllGather", ALU.bypass, replica_groups=RG,
                      ins=[t2_shard[:, :]], outs=[t2_full[:, :]])
              if "2" in PH:
                  edge_phase(2)

    nc.finalize()
    return nc


# --------------------------------------------------------------------------
_CACHE = {}


def run(inputs, trace=False):
    cfg, in_maps, slot_global = prep(**inputs)
    ckey = (cfg.key, os.environ.get("GAT_PHASES", "012"),
            os.environ.get("GAT_BUFS", "2"), os.environ.get("GAT_NWLIM"),
            os.environ.get("GAT_SCRATCH"), os.environ.get("GAT_GMAX"),
            os.environ.get("GAT_REPEAT"))
    nc = _CACHE.get(ckey)
    if nc is None:
        nc = build(cfg)
        _CACHE[ckey] = nc
    if os.environ.get("GAT_SIM"):
        from concourse.bass_interp import MultiCoreSim
        nc.insert_bir_kernel_barrier_sem_inc()
        sim = MultiCoreSim(nc, CORES, aliases={}, require_finite=False,
                           require_nnan=False)
        innames = [a.memorylocations[0].name
                   for a in nc.m.functions[0].allocations
                   if getattr(a, 'kind', None) == 'ExternalInput']
        for t in range(CORES):
            for nm in innames:
                if nm == 'partition_id':
                    sim.cores[t].tensor(nm)[:] = np.array([[t]], np.uint32)
                else:
                    sim.cores[t].tensor(nm)[:] = in_maps[t][nm]
        sim.simulate()
        class R: pass
        res = R(); res.results = [
            {"out": np.array(sim.cores[t].tensor("out"))}
            for t in range(CORES)]
        res.exec_time_ns = None
    else:
        res = run_bass_kernel_spmd(nc, in_maps, core_ids=list(range(CORES)),
                                   trace=trace)
    full = np.concatenate([res.results[c]["out"] for c in range(CORES)],
                          axis=0)
    return full[slot_global], res


# --------------------------------------------------------------------------
# harness entry point
# --------------------------------------------------------------------------

def kernel(**inputs):
    """Full unsharded inputs -> full [N, 128] output (runs on 8 NeuronCores)."""
    out, _ = run(inputs)
    return out
